# revision 1
# baseline (speedup 1.0000x reference)
"""DeformableFeatureAggregation on 8 Trainium2 NeuronCores (Bass/Tile), v2.

Sharding: 4096 anchors split across 8 cores (512 each); feature pyramids
(host-expanded into 2x2 bilinear-neighborhood rows) and projection weights
replicated; no collectives — each core writes its own 512 anchors.

v2 vs v1 device pipeline:
- corner FMA uses tensor_scalar (4x DVE perf mode) instead of
  scalar_tensor_tensor (no perf mode), with ~14/24 of the per-(cam,corner)
  multiplies offloaded to the ACT engine (Copy activation with per-partition
  scale); adds/group-weight ops batched over all 6 cams per op in bf16.
- softmax weights kept j-major in bf16 (no host column permute); W_wts bias
  folded into the logits matmul via a ones-row stationary trick.
- gathers merged: level0 cam-pairs (3 gathers) + one 768-idx gather per
  level 1..3 (single gathers must stay well under the SWDGE descriptor-ring
  capacity; a 2304-idx gather crashes the device).
- the (block, level, point) partial sums accumulate in PSUM f32 via PE
  identity matmuls, folded over cams at block end with one reduce.
"""

import sys
if '/opt/trn_rl_repo' not in sys.path:
    sys.path.insert(0, '/opt/trn_rl_repo')
import numpy as np
import ml_dtypes
from contextlib import ExitStack
import concourse.bass as bass
import concourse.tile as tile
from concourse import bacc, mybir

F32 = mybir.dt.float32
BF16 = mybir.dt.bfloat16
I16 = mybir.dt.int16
I32 = mybir.dt.int32
ALU = mybir.AluOpType
ACTF = mybir.ActivationFunctionType

NA = 512            # anchors per core
NB = 4              # anchor blocks of 128
PTS = 13
CAMS = 6
LVLS = 4
GRP = 8
ED = 256
NWTS = 2496         # 312*8
WL = [176, 88, 44, 22]
HL = [64, 32, 16, 8]
ROWS = [(h + 1) * (w + 1) for h, w in zip(HL, WL)]   # [11505, 2937, 765, 207]

# (corner k, cam c) pairs whose corner-multiply runs on ACT instead of DVE
ACT_KC = {(k, c) for k in (0, 1) for c in range(CAMS)} | {(2, 0)}


def build_program(skip_fma=False, skip_gather=False, reps=1):
    nc = bacc.Bacc("TRN2", target_bir_lowering=False, debug=False, num_devices=8)
    dram = {}
    def din(name, shape, dt):
        dram[name] = nc.dram_tensor(name, shape, dt, kind="ExternalInput").ap()
        return dram[name]

    anchor = din("anchor", [NA, 11], F32)
    ift32 = din("ift32", [ED, NA], F32)
    iftbf = din("iftbf", [ED, NA], BF16)
    ifeat = din("ifeat", [NA, ED], F32)
    E0 = din("E0", [CAMS * ROWS[0], 1024], BF16)
    E123 = din("E123", [CAMS * (ROWS[1] + ROWS[2] + ROWS[3]), 1024], BF16)
    wlearn = din("wlearn", [ED, 18], F32)
    blearn = din("blearn", [128, 18], F32)
    wwts = din("wwts", [ED, NWTS], BF16)
    bwrow = din("bwrow", [128, NWTS], BF16)   # row 0 = b_wts
    wout = din("wout", [ED, ED], F32)
    bout = din("bout", [128, ED], F32)
    fsc = din("fsc", [128, 21], F32)
    pmat = din("pmat", [128, 96], F32)
    invwh = din("invwh", [128, 12], F32)
    wrap = din("wrap", [128, 8 * 128], F32)
    ident = din("ident", [128, 128], F32)
    onesrow = din("onesrow", [128, 128], BF16)  # row 0 = ones (bias matmul)
    blc = din("blc", [128, LVLS * CAMS], F32)   # per-(level,cam) idx bases
    out_d = nc.dram_tensor("out", [NA, ED], F32, kind="ExternalOutput").ap()

    with tile.TileContext(nc) as tc, ExitStack() as ctx:
        cpool = ctx.enter_context(tc.tile_pool(name="const", bufs=1))
        apool = ctx.enter_context(tc.tile_pool(name="aout", bufs=1))
        wpool = ctx.enter_context(tc.tile_pool(name="work", bufs=2))
        tpool = ctx.enter_context(tc.tile_pool(name="tmp13", bufs=2))
        glpool = ctx.enter_context(tc.tile_pool(name="gl0", bufs=2))
        g123pool = ctx.enter_context(tc.tile_pool(name="g123", bufs=2))
        t6pool = ctx.enter_context(tc.tile_pool(name="t6", bufs=2))
        ipool = ctx.enter_context(tc.tile_pool(name="idx16", bufs=2))
        scpool = ctx.enter_context(tc.tile_pool(name="stc", bufs=1))
        ps_wp = ctx.enter_context(tc.tile_pool(name="psA", bufs=2, space="PSUM"))
        ps_s6 = ctx.enter_context(tc.tile_pool(name="psS6", bufs=1, space="PSUM"))
        ps_sm = ctx.enter_context(tc.tile_pool(name="psS", bufs=1, space="PSUM"))
        ps_w = ctx.enter_context(tc.tile_pool(name="psW", bufs=2, space="PSUM"))

        def ctile(ap_src, shape, dt, tag):
            t = cpool.tile(shape, dt, tag=tag)
            nc.sync.dma_start(t[:], ap_src)
            return t

        # ---- persistent constants in SBUF ----
        ift32_t = [ctile(ift32[k * 128:(k + 1) * 128, :], [128, NA], F32, f"ift32_{k}") for k in range(2)]
        iftbf_t = [ctile(iftbf[k * 128:(k + 1) * 128, :], [128, NA], BF16, f"iftbf_{k}") for k in range(2)]
        wl_t = [ctile(wlearn[k * 128:(k + 1) * 128, :], [128, 18], F32, f"wl_{k}") for k in range(2)]
        bl_t = ctile(blearn[:, :], [128, 18], F32, "bl")
        ww_t = [ctile(wwts[k * 128:(k + 1) * 128, :], [128, NWTS], BF16, f"ww_{k}") for k in range(2)]
        bwr_t = ctile(bwrow[:, :], [128, NWTS], BF16, "bwr")
        wo_t = [ctile(wout[k * 128:(k + 1) * 128, :], [128, ED], F32, f"wo_{k}") for k in range(2)]
        bo_t = ctile(bout[:, :], [128, ED], F32, "bo")
        fs_t = ctile(fsc[:, :], [128, 21], F32, "fs")
        pm_t = ctile(pmat[:, :], [128, 96], F32, "pm")
        iw_t = ctile(invwh[:, :], [128, 12], F32, "iw")
        wrap_t = ctile(wrap[:, :], [128, 8, 128], F32, "wrap")
        id_t = ctile(ident[:, :], [128, 128], F32, "id")
        idb_t = cpool.tile([128, 128], BF16, tag="idb")
        V0 = nc.vector
        V0.tensor_copy(idb_t[:], id_t[:])
        or_t = ctile(onesrow[:, :], [128, 128], BF16, "or")
        blc_t = ctile(blc[:, :], [128, LVLS, CAMS], F32, "blc")

        # ---- stage A persistent outputs (all blocks) ----
        # weights j-major: [c, l*13+pt, g]
        wn_b = [apool.tile([128, CAMS, 52, GRP], BF16, tag=f"wn{b}", name=f"wn{b}") for b in range(NB)]
        # corner weights: [l, k, c, pt]
        bwp_b = [apool.tile([128, LVLS, 4, CAMS, PTS], F32, tag=f"bwp{b}", name=f"bwp{b}") for b in range(NB)]
        idxf_b = [apool.tile([128, LVLS, CAMS, PTS], F32, tag=f"idxf{b}", name=f"idxf{b}") for b in range(NB)]
        S_b = [apool.tile([128, ED], F32, tag=f"S{b}", name=f"S{b}") for b in range(NB)]
        sinv_b = [apool.tile([128, GRP], F32, tag=f"si{b}", name=f"si{b}") for b in range(NB)]

        V = nc.vector
        SC = nc.scalar
        gdum = None
        if skip_gather:
            gdum = cpool.tile([128, CAMS, 1024], BF16, tag="gdum", name="gdum")
            nc.vector.memset(gdum[:], 0.0)

        for _rep in range(reps):
            # =============== STAGE A ===============
            for b in range(NB):
                a0 = b * 128
                anc = wpool.tile([128, 11], F32, tag="anc", name="anc")
                nc.sync.dma_start(anc[:], anchor[a0:a0 + 128, :])

                # learnable scale: L = IF @ W_learn + b_learn ; sigmoid(L) - 0.5
                # sigmoid via exp (keeps ACT on the exp_and_others table set):
                # s = 1/(1+exp(-L))
                Lp = ps_sm.tile([128, 18], F32, tag="pss", name="Lp")
                for k in range(2):
                    nc.tensor.matmul(Lp[:], ift32_t[k][:, a0:a0 + 128], wl_t[k][:],
                                     start=(k == 0), stop=(k == 1))
                Lb = wpool.tile([128, 18], F32, tag="Lb", name="Lb")
                V.tensor_add(Lb[:], Lp[:], bl_t[:])
                eL = wpool.tile([128, 18], F32, tag="eL", name="eL")
                SC.activation(eL[:], Lb[:], ACTF.Exp, scale=-1.0)
                V.tensor_scalar_add(eL[:], eL[:], 1.0)
                V.reciprocal(eL[:], eL[:])
                scale3 = wpool.tile([128, 39], F32, tag="scale3", name="scale3")
                V.tensor_copy(scale3[:, 0:21], fs_t[:])
                V.tensor_scalar_sub(scale3[:, 21:39], eL[:], 0.5)

                # kp0 = scale3 * gs_scales (broadcast xyz)
                kp0 = wpool.tile([128, PTS, 3], F32, tag="kp0", name="kp0")
                gs_b = anc[:, 3:6].unsqueeze(1).broadcast_to([128, PTS, 3])
                V.tensor_tensor(kp0[:], scale3[:].rearrange("p (t x) -> p t x", x=3), gs_b, ALU.mult)

                # rotation from quaternion
                qq = wpool.tile([128, 16], F32, tag="qq", name="qq")
                q_i = anc[:, 6:10].unsqueeze(2).broadcast_to([128, 4, 4])
                q_j = anc[:, 6:10].unsqueeze(1).broadcast_to([128, 4, 4])
                V.tensor_tensor(qq[:].rearrange("p (i j) -> p i j", i=4), q_i, q_j, ALU.mult)
                sq4 = wpool.tile([128, 4], F32, tag="sq4", name="sq4")
                V.tensor_tensor(sq4[:], anc[:, 6:10], anc[:, 6:10], ALU.mult)
                n2 = wpool.tile([128, 1], F32, tag="n2", name="n2")
                V.tensor_reduce(n2[:], sq4[:], mybir.AxisListType.X, ALU.add)
                inv2 = wpool.tile([128, 1], F32, tag="inv2", name="inv2")
                V.reciprocal(inv2[:], n2[:])
                V.tensor_scalar_mul(qq[:], qq[:], inv2[:, 0:1])
                R = wpool.tile([128, 9], F32, tag="R", name="R")
                tmp1 = wpool.tile([128, 1], F32, tag="tmp1", name="tmp1")
                for i, (ca, cb) in enumerate([(10, 15), (5, 15), (5, 10)]):
                    V.tensor_add(tmp1[:], qq[:, ca:ca + 1], qq[:, cb:cb + 1])
                    V.tensor_scalar(R[:, 4 * i:4 * i + 1], tmp1[:], -2.0, 1.0, ALU.mult, ALU.add)
                for (ca, cb, op, d) in [(6, 3, ALU.subtract, 1), (7, 2, ALU.add, 2),
                                        (6, 3, ALU.add, 3), (11, 1, ALU.subtract, 5),
                                        (7, 2, ALU.subtract, 6), (11, 1, ALU.add, 7)]:
                    V.tensor_tensor(tmp1[:], qq[:, ca:ca + 1], qq[:, cb:cb + 1], op)
                    V.tensor_scalar_mul(R[:, d:d + 1], tmp1[:], 2.0)

                # kp rotated + translated
                kpr = wpool.tile([128, 3, PTS], F32, tag="kpr", name="kpr")
                t13a = tpool.tile([128, PTS], F32, tag="t13a", name="t13a")
                for i in range(3):
                    V.tensor_scalar_mul(t13a[:], kp0[:, :, 0], R[:, i:i + 1])
                    V.scalar_tensor_tensor(t13a[:], kp0[:, :, 1], R[:, 3 + i:4 + i], t13a[:], ALU.mult, ALU.add)
                    V.scalar_tensor_tensor(t13a[:], kp0[:, :, 2], R[:, 6 + i:7 + i], t13a[:], ALU.mult, ALU.add)
                    V.tensor_scalar_add(kpr[:, i, :], t13a[:], anc[:, i:i + 1])

                # projection per cam -> xh/yh in [0, 0.9999]
                xh = wpool.tile([128, CAMS, PTS], F32, tag="xh", name="xh")
                yh = wpool.tile([128, CAMS, PTS], F32, tag="yh", name="yh")
                for c in range(CAMS):
                    pc = c * 16
                    rows = []
                    for i in range(3):
                        ti = tpool.tile([128, PTS], F32, tag=f"proj{i}", name=f"proj{i}")
                        V.tensor_scalar_mul(ti[:], kpr[:, 0, :], pm_t[:, pc + 4 * i:pc + 4 * i + 1])
                        V.scalar_tensor_tensor(ti[:], kpr[:, 1, :], pm_t[:, pc + 4 * i + 1:pc + 4 * i + 2], ti[:], ALU.mult, ALU.add)
                        V.scalar_tensor_tensor(ti[:], kpr[:, 2, :], pm_t[:, pc + 4 * i + 2:pc + 4 * i + 3], ti[:], ALU.mult, ALU.add)
                        V.tensor_scalar_add(ti[:], ti[:], pm_t[:, pc + 4 * i + 3:pc + 4 * i + 4])
                        rows.append(ti)
                    X, Y, Z = rows
                    V.tensor_single_scalar(Z[:], Z[:], 1e-5, ALU.max)
                    Zi = tpool.tile([128, PTS], F32, tag="Zi", name="Zi")
                    V.reciprocal(Zi[:], Z[:])
                    for (src, dst, col) in [(X, xh, 2 * c), (Y, yh, 2 * c + 1)]:
                        V.tensor_tensor(t13a[:], src[:], Zi[:], ALU.mult)
                        V.tensor_scalar_mul(t13a[:], t13a[:], iw_t[:, col:col + 1])
                        V.tensor_scalar(dst[:, c, :], t13a[:], 0.0, 0.9999, ALU.max, ALU.min)

                # per level (all 6 cams batched): bilinear weights + cell index
                p78 = tpool.tile([128, CAMS, PTS], F32, tag="p78", name="p78")
                q78 = tpool.tile([128, CAMS, PTS], F32, tag="q78", name="q78")
                wx6 = tpool.tile([128, CAMS, PTS], F32, tag="wx6", name="wx6")
                wy6 = tpool.tile([128, CAMS, PTS], F32, tag="wy6", name="wy6")
                ox6 = tpool.tile([128, CAMS, PTS], F32, tag="ox6", name="ox6")
                oy6 = tpool.tile([128, CAMS, PTS], F32, tag="oy6", name="oy6")
                xb6 = tpool.tile([128, CAMS, PTS], F32, tag="xb6", name="xb6")
                yb6 = tpool.tile([128, CAMS, PTS], F32, tag="yb6", name="yb6")
                ifx6 = tpool.tile([128, CAMS, PTS], I32, tag="ifx6", name="ifx6")
                t78 = tpool.tile([128, CAMS, PTS], F32, tag="t78", name="t78")
                for l in range(LVLS):
                    V.tensor_scalar(p78[:], xh[:], float(WL[l]), 0.5, ALU.mult, ALU.add)
                    V.tensor_scalar(q78[:], yh[:], float(HL[l]), 0.5, ALU.mult, ALU.add)
                    for (p1, bt, wt) in ((p78, xb6, wx6), (q78, yb6, wy6)):
                        V.tensor_copy(ifx6[:], p1[:])          # f32 -> int32 (round-to-nearest-ish)
                        V.tensor_copy(bt[:], ifx6[:])          # back to f32
                        V.tensor_sub(wt[:], p1[:], bt[:])      # err (may be <0)
                        V.tensor_single_scalar(t78[:], wt[:], 0.0, ALU.is_lt)
                        V.tensor_sub(bt[:], bt[:], t78[:])     # floor
                        V.tensor_sub(wt[:], p1[:], bt[:])      # frac in [0,1)
                    V.tensor_scalar(ox6[:], wx6[:], -1.0, 1.0, ALU.mult, ALU.add)
                    V.tensor_scalar(oy6[:], wy6[:], -1.0, 1.0, ALU.mult, ALU.add)
                    # corners: 0:(y0,x0) 1:(y0,x1) 2:(y1,x0) 3:(y1,x1)
                    V.tensor_tensor(bwp_b[b][:, l, 0], oy6[:], ox6[:], ALU.mult)
                    V.tensor_tensor(bwp_b[b][:, l, 1], oy6[:], wx6[:], ALU.mult)
                    V.tensor_tensor(bwp_b[b][:, l, 2], wy6[:], ox6[:], ALU.mult)
                    V.tensor_tensor(bwp_b[b][:, l, 3], wy6[:], wx6[:], ALU.mult)
                    # cell index: yb*(W+1)+xb + base(l,c)
                    V.scalar_tensor_tensor(t78[:], yb6[:], float(WL[l] + 1), xb6[:], ALU.mult, ALU.add)
                    V.tensor_tensor(idxf_b[b][:, l], t78[:],
                                    blc_t[:, l, :].unsqueeze(2).broadcast_to([128, CAMS, PTS]), ALU.add)

                # weights: logits = IF @ W_wts + b_wts (ones-row trick); softmax
                # over (c, l, pt) per group; kept j-major bf16
                wnb = wn_b[b]
                wn_flat = wnb[:].rearrange("p c j g -> p (c j g)")
                for n0 in range(0, NWTS, 512):
                    n1 = min(n0 + 512, NWTS)
                    Wp = ps_wp.tile([128, 512], F32, tag="Wp", name="Wp")
                    for k in range(2):
                        nc.tensor.matmul(Wp[:, 0:n1 - n0], iftbf_t[k][:, a0:a0 + 128], ww_t[k][:, n0:n1],
                                         start=(k == 0), stop=False)
                    nc.tensor.matmul(Wp[:, 0:n1 - n0], or_t[:], bwr_t[:, n0:n1],
                                     start=False, stop=True)
                    SC.activation(wn_flat[:, n0:n1], Wp[:, 0:n1 - n0], ACTF.Exp)
                ssum = wpool.tile([128, GRP], F32, tag="ssum", name="ssum")
                # per-group sums on ACT (accum_out) — (c j) merges to a
                # uniform stride-8 view; in-place identity copy, sum captured
                wng = wnb[:].rearrange("p c j g -> p g (c j)")
                for g in range(GRP):
                    SC.activation(wng[:, g, :], wng[:, g, :], ACTF.Copy,
                                  accum_out=ssum[:, g:g + 1])
                V.reciprocal(sinv_b[b][:], ssum[:])

            # =============== STAGE B: gather + FMA ===============
            for b in range(NB):
                S6p = ps_s6.tile([128, CAMS * ED], F32, tag="S6p", name="S6p")

                def wrap_idx(pt):
                    # wrap indices into the 16-partition gather layout via PE
                    mv = idxf_b[b][:, :, :, pt].rearrange("p l c -> p (l c)")
                    P8 = ps_w.tile([128, 8, 24], F32, tag="pw", name="P8")
                    for s in range(8):
                        nc.tensor.matmul(P8[:, s, :], wrap_t[:, s, :], mv, start=True, stop=True)
                    i16 = ipool.tile([128, 24, 8], I16, tag="i16", name="i16")
                    V.tensor_copy(i16[:].rearrange("p j s -> p s j"), P8[:])
                    return i16[:].rearrange("p j s -> p (j s)")

                i16_next = wrap_idx(0)
                for pt in range(PTS):
                    i16f = i16_next

                    if skip_gather:
                        gl0 = g123 = None
                    else:
                        gl0 = glpool.tile([128, CAMS, 1024], BF16, tag="gl0", name="gl0")
                        for p in range(3):   # level-0 cam pairs
                            nc.gpsimd.dma_gather(
                                out_ap=gl0[:, 2 * p:2 * p + 2, :],
                                in_ap=E0[p * 2 * ROWS[0]:(p + 1) * 2 * ROWS[0], :],
                                idxs_ap=i16f[:, 16 * p:16 * p + 16],
                                num_idxs=256, num_idxs_reg=256, elem_size=1024)
                        g123 = g123pool.tile([128, 3 * CAMS, 1024], BF16, tag="g123", name="g123")
                        for li in range(3):   # one 768-idx gather per level 1..3
                            nc.gpsimd.dma_gather(
                                out_ap=g123[:, 6 * li:6 * (li + 1), :], in_ap=E123[:, :],
                                idxs_ap=i16f[:, 48 * (li + 1):48 * (li + 2)],
                                num_idxs=768, num_idxs_reg=768, elem_size=1024)

                    if pt + 1 < PTS:
                        i16_next = wrap_idx(pt + 1)
                    if skip_fma:
                        continue
                    for l in range(LVLS):
                        if skip_gather:
                            G6 = gdum[:, :, :]
                        elif l == 0:
                            G6 = gl0[:, :, :]
                        else:
                            G6 = g123[:, 6 * (l - 1):6 * l, :]
                        # corner multiplies in place over the gather tile (each
                        # 256-chunk is consumed exactly once); DVE corners (k2,k3)
                        # first so the add tree starts without waiting on ACT
                        for k in (2, 3, 0, 1):
                            for c in range(CAMS):
                                src = G6[:, c, 256 * k:256 * (k + 1)]
                                sc = bwp_b[b][:, l, k, c, pt:pt + 1]
                                if (k, c) in ACT_KC:
                                    SC.activation(src, src, ACTF.Copy, scale=sc)
                                else:
                                    V.tensor_scalar_mul(src, src, sc)
                        T6 = t6pool.tile([128, CAMS, ED], BF16, tag="T6", name="T6")
                        V.tensor_add(T6[:], G6[:, :, 512:768], G6[:, :, 768:1024])
                        V.tensor_add(T6[:], T6[:], G6[:, :, 0:256])
                        V.tensor_add(T6[:], T6[:], G6[:, :, 256:512])
                        # group weights (j-major bf16) in place, then accumulate
                        # into PSUM via PE identity matmuls (f32 accumulation)
                        wv6 = wn_b[b][:, :, l * PTS + pt, :].unsqueeze(2).broadcast_to([128, CAMS, 32, GRP])
                        t6g = T6[:].rearrange("p c (d g) -> p c d g", g=GRP)
                        V.tensor_tensor(t6g, t6g, wv6, ALU.mult)
                        t6f = T6[:].rearrange("p c e -> p (c e)")
                        first = (pt == 0 and l == 0)
                        last = (pt == PTS - 1 and l == LVLS - 1)
                        for h in range(3):
                            nc.tensor.matmul(S6p[:, 512 * h:512 * (h + 1)], idb_t[:],
                                             t6f[:, 512 * h:512 * (h + 1)],
                                             start=first, stop=last)

                # fold PSUM partial into f32 per-block sum over cams
                if skip_fma:
                    V.memset(S_b[b][:], 0.0)
                else:
                    V.tensor_reduce(S_b[b][:],
                                    S6p[:].rearrange("p (c ch) -> p ch c", c=CAMS),
                                    mybir.AxisListType.X, ALU.add)
                    V.tensor_tensor(S_b[b][:].rearrange("p (d g) -> p d g", g=GRP),
                                    S_b[b][:].rearrange("p (d g) -> p d g", g=GRP),
                                    sinv_b[b][:].unsqueeze(1).broadcast_to([128, 32, GRP]),
                                    ALU.mult)

            # =============== STAGE C: output projection ===============
            for b in range(NB):
                a0 = b * 128
                FT = scpool.tile([128, 2, 128], F32, tag="FT", name="FT")
                for k in range(2):
                    pt_ = ps_sm.tile([128, 128], F32, tag="pss", name="ptT")
                    nc.tensor.transpose(pt_[:], S_b[b][:, 128 * k:128 * (k + 1)], id_t[:])
                    V.tensor_copy(FT[:, k, :], pt_[:])
                Op = ps_sm.tile([128, ED], F32, tag="pss", name="Op")
                for k in range(2):
                    nc.tensor.matmul(Op[:], FT[:, k, :], wo_t[k][:], start=(k == 0), stop=(k == 1))
                ifl = scpool.tile([128, ED], F32, tag="ifl", name="ifl")
                nc.sync.dma_start(ifl[:], ifeat[a0:a0 + 128, :])
                Ob = scpool.tile([128, ED], F32, tag="Ob", name="Ob")
                V.tensor_add(Ob[:], Op[:], bo_t[:])
                V.tensor_add(Ob[:], Ob[:], ifl[:])
                nc.sync.dma_start(out_d[a0:a0 + 128, :], Ob[:])

    nc.compile()
    return nc


# channel permutation: new position d*8+g holds original channel g*32+d
CH_PERM = np.array([g * 32 + d for d in range(32) for g in range(8)], dtype=np.int64)


def host_prep(inputs):
    """Build per-core in_maps from full inputs dict."""
    IF = np.asarray(inputs["instance_feature"][0])      # [4096, 256]
    AN = np.asarray(inputs["anchor"][0])                # [4096, 11]
    PM = np.asarray(inputs["projection_mat"][0])        # [6, 4, 4]
    IW = np.asarray(inputs["image_wh"][0])              # [6, 2]
    Wl = np.asarray(inputs["W_learn"]); bl = np.asarray(inputs["b_learn"])
    Ww = np.asarray(inputs["W_wts"]); bw = np.asarray(inputs["b_wts"])
    Wo = np.asarray(inputs["W_out"]); bo = np.asarray(inputs["b_out"])
    feats = [np.asarray(inputs[f"feat{l}"][0]) for l in range(4)]  # [6, 256, H, W]

    ones = np.ones((128, 1), np.float32)
    # expanded neighborhood maps, channel-permuted, bf16
    Emaps = []
    for l, fm in enumerate(feats):
        fmp = fm[:, CH_PERM]                             # [6, 256, H, W] perm'd
        H, W = HL[l], WL[l]
        Mp = np.zeros((CAMS, H + 2, W + 2, 256), np.float32)
        Mp[:, 1:H + 1, 1:W + 1, :] = np.transpose(fmp, (0, 2, 3, 1))
        Eh = np.concatenate([Mp[:, :-1, :-1], Mp[:, :-1, 1:], Mp[:, 1:, :-1], Mp[:, 1:, 1:]], axis=-1)
        Emaps.append(np.ascontiguousarray(Eh.reshape(CAMS * ROWS[l], 1024)).astype(ml_dtypes.bfloat16))

    Wo_p = np.ascontiguousarray(Wo[CH_PERM, :]).astype(np.float32)

    FIX = np.array([[0, 0, 0], [0.45, 0, 0], [-0.45, 0, 0], [0, 0.45, 0],
                    [0, -0.45, 0], [0, 0, 0.45], [0, 0, -0.45]], np.float32)

    wrap_h = np.zeros((128, 8, 128), np.float32)
    for s in range(8):
        for m in range(128):
            wrap_h[s * 16 + (m % 16), s, m] = 1.0

    onesrow_h = np.zeros((128, 128), np.float32)
    onesrow_h[0, :] = 1.0

    # idx bases per (level, cam): l0 cam-pairs; E1 per-cam; E23 = [E2|E3]
    blc_h = np.zeros((LVLS, CAMS), np.float32)
    for c in range(CAMS):
        blc_h[0, c] = (c % 2) * ROWS[0]
        blc_h[1, c] = c * ROWS[1]
        blc_h[2, c] = CAMS * ROWS[1] + c * ROWS[2]
        blc_h[3, c] = CAMS * (ROWS[1] + ROWS[2]) + c * ROWS[3]

    shared = {
        "wlearn": Wl.astype(np.float32), "blearn": ones * 0 + bl[None, :].astype(np.float32),
        "wwts": Ww.astype(ml_dtypes.bfloat16),
        "bwrow": np.tile(bw[None, :].astype(np.float32), (128, 1)).astype(ml_dtypes.bfloat16),
        "wout": Wo_p, "bout": ones * 0 + bo[None, :].astype(np.float32),
        "fsc": np.tile(FIX.reshape(1, 21), (128, 1)).astype(np.float32),
        "pmat": np.tile(PM.reshape(1, 96), (128, 1)).astype(np.float32),
        "invwh": np.tile((1.0 / IW).reshape(1, 12), (128, 1)).astype(np.float32),
        "wrap": wrap_h.reshape(128, 8 * 128),
        "ident": np.eye(128, dtype=np.float32),
        "onesrow": onesrow_h.astype(ml_dtypes.bfloat16),
        "blc": np.tile(blc_h.reshape(1, LVLS * CAMS), (128, 1)).astype(np.float32),
        "E0": Emaps[0],
        "E123": np.concatenate([Emaps[1], Emaps[2], Emaps[3]], axis=0),
    }

    in_maps = []
    for core in range(8):
        s = slice(core * NA, (core + 1) * NA)
        m = dict(shared)
        m["anchor"] = np.ascontiguousarray(AN[s]).astype(np.float32)
        ifc = np.ascontiguousarray(IF[s]).astype(np.float32)
        m["ifeat"] = ifc
        ift = np.ascontiguousarray(ifc.T)
        m["ift32"] = ift
        m["iftbf"] = ift.astype(ml_dtypes.bfloat16)
        in_maps.append(m)
    return in_maps


def assemble(results):
    return np.concatenate([results[c]["out"] for c in range(8)], axis=0)[None]


def kernel(**inputs):
    from concourse.bass_utils import run_bass_kernel_spmd
    nc = build_program()
    in_maps = host_prep(inputs)
    res = run_bass_kernel_spmd(nc, in_maps, list(range(8))).results
    return assemble(res).astype(np.float32)

